# revision 3
# baseline (speedup 1.0000x reference)
"""Trainium2 Bass kernel for nn_DihedralAngleLayer (planar v2).

Input:  x [2_000_000, 42] f32 (14 atoms x 3 coords per row),
        mask_matrix [4, 14] f32 one-hot carbon selector.
Output: dihedral angle per row, [2_000_000] f32.

Data-parallel across 8 NeuronCores; rows padded to 8*128*1960 and split.
Per core, rows are partition-major: partition p owns rows [p*Q, (p+1)*Q).

Layout: Q = T*G columns per partition, T=10 uniform blocks of G=196.
Per block, two fused transposing subtracts turn the row-major [G, 42] tile
into 9 G-length bond-vector mini-planes (u=c1-c0, v=c2-c1, w=c3-c2 per
component).  All downstream math then runs as full-rate unit-stride plane
ops (f32 TENSOR_TENSOR has no DVE fast mode; interleaved runs-of-3 cost
~1.3x and single-elem strided runs ~2x, measured):

    NB = v x w;  NA = u x v                (pairwise plane mults, P1-P2)
    xx = NA.NB;  yy = sqrt(v.v) * (u.NB)
    phi = half-angle atan2:  t = yy/(rho+|xx|), rho = sqrt(xx^2+yy^2)
          phi = 2*arctan(t)*sgx + (pi/2)*(sgy - sgx*sgy)

Squares/sqrt/abs/arctan/sign run on the Scalar(ACT) engine.  Input DMAs on
nc.sync (HWDGE), output on nc.scalar: GPSIMD stays idle so SWDGE descriptor
generation never contends with DVE.  Chunks of [1,1,1,1,2,2,2] blocks
pipeline phase 2 against the DMA stream (small early chunks start DVE
sooner; small-ish late chunks bound the post-last-DMA exposure).
"""

import numpy as np

import concourse.bacc as bacc
import concourse.bass as bass
import concourse.mybir as mybir
from concourse.bass_utils import run_bass_kernel_spmd
from concourse.tile import TileContext

AF = mybir.ActivationFunctionType
OP = mybir.AluOpType
F32 = mybir.dt.float32

PI = float(np.pi)

N_CORES = 8
G = 196
T = 10
Q = G * T                   # 1960 rows per partition
ROWS_PER_CORE = 128 * Q     # 250880
CHUNKS = [1, 1, 1, 1, 2, 2, 2]

BS = 9 * G                  # b-plane block stride (9 mini-planes per block)
SP = 13                     # scratch planes per block
SS = SP * G                 # scratch block stride


def _ap(base, off, dims):
    return bass.AP(
        base.tensor, base.offset + off, [list(base.ap[0])] + [list(d) for d in dims]
    )


def _emit_ph1(nc, bba, xta, b, c0, c1, c2, c3):
    """Two fused transposing subtracts: row-major tile -> 9 G-long planes."""
    v = nc.vector
    # (u, v) = (c1-c0, c2-c1): dst collapses to one contiguous 6G run
    v.tensor_tensor(
        _ap(bba, b * BS, [[3 * G, 2], [G, 3], [1, G]]),
        _ap(xta, c1, [[c2 - c1, 2], [1, 3], [42, G]]),
        _ap(xta, c0, [[c1 - c0, 2], [1, 3], [42, G]]),
        OP.subtract,
    )
    # w = c3-c2
    v.tensor_tensor(
        _ap(bba, b * BS + 6 * G, [[G, 3], [1, G]]),
        _ap(xta, c3, [[1, 3], [42, G]]),
        _ap(xta, c2, [[1, 3], [42, G]]),
        OP.subtract,
    )


def _emit_ph2(nc, bba, sca, oba, y, b0, nb):
    """Cross-form dihedral + half-angle atan2 for chunk [b0, b0+nb) blocks."""
    v, s = nc.vector, nc.scalar

    def bap(plane, n=1, ps=1):
        dims = [[BS, nb]] + ([[ps * G, n]] if n > 1 else []) + [[1, G]]
        return _ap(bba, b0 * BS + plane * G, dims)

    def sap(plane, n=1, ps=1):
        dims = [[SS, nb]] + ([[ps * G, n]] if n > 1 else []) + [[1, G]]
        return _ap(sca, plane * G, dims)

    # q3 = v*v (ACT), q = sum
    s.activation(sap(0, 3), bap(3, 3), AF.Square)
    v.tensor_tensor(sap(9), sap(0), sap(1), OP.add)
    v.tensor_tensor(sap(9), sap(9), sap(2), OP.add)
    # NB = v x w: P1 = (vy,vz,vx)*(wz,wx,wy), P2 = (vz,vx,vy)*(wy,wz,wx)
    v.tensor_tensor(sap(0, 2), bap(4, 2), bap(8, 2, -2), OP.mult)
    v.tensor_tensor(sap(2), bap(3), bap(7), OP.mult)
    v.tensor_tensor(sap(3, 2), bap(5, 2, -2), bap(7, 2), OP.mult)
    v.tensor_tensor(sap(5), bap(4), bap(6), OP.mult)
    v.tensor_tensor(sap(6, 3), sap(0, 3), sap(3, 3), OP.subtract)
    # det = u.NB
    v.tensor_tensor(sap(0, 3), bap(0, 3), sap(6, 3), OP.mult)
    v.tensor_tensor(sap(0), sap(0), sap(1), OP.add)
    v.tensor_tensor(sap(0), sap(0), sap(2), OP.add)
    # yy = det * sqrt(q)
    s.activation(sap(1), sap(9), AF.Sqrt)
    v.tensor_tensor(sap(11), sap(0), sap(1), OP.mult)
    # NA = u x v
    v.tensor_tensor(sap(3, 2), bap(1, 2), bap(5, 2, -2), OP.mult)
    v.tensor_tensor(sap(5), bap(0), bap(4), OP.mult)
    v.tensor_tensor(sap(0, 2), bap(2, 2, -2), bap(4, 2), OP.mult)
    v.tensor_tensor(sap(2), bap(1), bap(3), OP.mult)
    v.tensor_tensor(sap(3, 3), sap(3, 3), sap(0, 3), OP.subtract)
    # xx = NA.NB
    v.tensor_tensor(sap(6, 3), sap(3, 3), sap(6, 3), OP.mult)
    v.tensor_tensor(sap(10), sap(6), sap(7), OP.add)
    v.tensor_tensor(sap(10), sap(10), sap(8), OP.add)
    # tail: half-angle atan2(yy, xx)
    s.activation(sap(12), sap(10), AF.Abs)             # ax
    s.activation(sap(0, 2), sap(10, 2), AF.Square)     # xx^2, yy^2
    v.tensor_tensor(sap(0), sap(0), sap(1), OP.add)    # rho^2
    s.activation(sap(1), sap(0), AF.Sqrt)              # rho
    v.tensor_tensor(sap(1), sap(1), sap(12), OP.add)   # d = rho + ax
    v.reciprocal_approx_fast(sap(2), sap(1))           # rd
    v.tensor_tensor(sap(0), sap(11), sap(2), OP.mult)  # t = yy * rd
    s.activation(sap(1), sap(0), AF.Arctan)            # A
    s.activation(sap(3), sap(10), AF.Sign)             # sgx
    s.activation(sap(4), sap(11), AF.Sign)             # sgy
    v.tensor_tensor(sap(5), sap(3), sap(4), OP.mult)   # s2 = sgx*sgy
    v.tensor_tensor(sap(4), sap(4), sap(5), OP.subtract)  # t2 = sgy - s2
    v.scalar_tensor_tensor(sap(3), sap(3), 2.0, sap(1), OP.mult, OP.mult)  # 2*A*sgx
    v.scalar_tensor_tensor(
        _ap(oba, 0, [[G, nb], [1, G]]), sap(4), PI / 2, sap(3), OP.mult, OP.add
    )
    nc.scalar.dma_start(
        out=y.rearrange("(p q) -> p q", p=128)[:, b0 * G : (b0 + nb) * G],
        in_=_ap(oba, 0, [[1, nb * G]]),
    )


def build_kernel(atoms):
    c0, c1, c2, c3 = (3 * int(a) for a in atoms)
    nc = bacc.Bacc("TRN2", target_bir_lowering=False, debug=False)
    x = nc.dram_tensor("x", [ROWS_PER_CORE, 42], F32, kind="ExternalInput")
    y = nc.dram_tensor("y", [ROWS_PER_CORE], F32, kind="ExternalOutput")
    xr = x.rearrange("(p q) c -> p q c", p=128)
    with TileContext(nc) as tc:
        with (
            tc.tile_pool(name="xp", bufs=3) as xp,
            tc.tile_pool(name="bp", bufs=1) as bp,
            tc.tile_pool(name="sp", bufs=1) as sp,
            tc.tile_pool(name="op", bufs=2) as op,
        ):
            bb = bp.tile([128, 9 * G * T], F32, tag="b")
            bba = bb[:]
            b = 0
            for nb in CHUNKS:
                for _ in range(nb):
                    xt = xp.tile([128, G * 42], F32, tag="x")
                    nc.sync.dma_start(out=xt[:], in_=xr[:, b * G : (b + 1) * G, :])
                    _emit_ph1(nc, bba, xt[:], b, c0, c1, c2, c3)
                    b += 1
                sc = sp.tile([128, SS * 2], F32, tag="sc")
                ob = op.tile([128, G * 2], F32, tag="o")
                _emit_ph2(nc, bba, sc[:], ob[:], y, b - nb, nb)
    nc.finalize()
    return nc


_CACHE = {}


def _get_nc(atoms):
    key = tuple(int(a) for a in atoms)
    if key not in _CACHE:
        _CACHE[key] = build_kernel(key)
    return _CACHE[key]


def run(x, atoms=(0, 4, 7, 11), **spmd_kwargs):
    """x: [B, 42] f32. Returns (y [B] f32, BassKernelResults)."""
    x = np.ascontiguousarray(np.asarray(x, dtype=np.float32))
    B = x.shape[0]
    total = N_CORES * ROWS_PER_CORE
    if B < total:
        # pad with replicated leading rows (valid, non-degenerate data)
        reps = -(-(total - B) // B)
        x = np.concatenate([x] + [x] * reps, axis=0)[:total]
    nc = _get_nc(atoms)
    shards = x.reshape(N_CORES, ROWS_PER_CORE, 42)
    in_maps = [{"x": shards[i]} for i in range(N_CORES)]
    res = run_bass_kernel_spmd(nc, in_maps, core_ids=list(range(N_CORES)), **spmd_kwargs)
    y = np.concatenate([r["y"] for r in res.results])[:B]
    return np.asarray(y, dtype=np.float32), res


def kernel(x, mask_matrix):
    mask = np.asarray(mask_matrix)
    atoms = tuple(int(i) for i in np.argmax(mask, axis=1))
    y, _ = run(x, atoms=atoms)
    return y


# revision 4
# speedup vs baseline: 1.2481x; 1.2481x over previous
"""Trainium2 Bass kernel for nn_DihedralAngleLayer (planar v3).

Input:  x [2_000_000, 42] f32 (14 atoms x 3 coords per row),
        mask_matrix [4, 14] f32 one-hot carbon selector.
Output: dihedral angle per row, [2_000_000] f32.

Data-parallel across 8 NeuronCores; rows padded to 8*128*1960 and split.
Per core, rows are partition-major: partition p owns rows [p*Q, (p+1)*Q).

Per uniform block (G=196 rows/partition) the row-major tile is planarized by
TWO transposing copies on the Scalar/ACT engine (carbon pairs (0,4) and
(7,11) both have uniform column stride, so each copy moves 2 carbons x 3
components in one 3-dim-AP op), then ONE contiguous DVE subtract forms all
nine bond-vector planes at once: (u,v,w) = carbons[1:4] - carbons[0:3].
This keeps the expensive strided access patterns off the Vector engine
(f32 TT has no DVE fast mode; strided single-element runs cost ~2x).

Phase 2 per chunk of blocks (planar, unit-stride, 18 DVE instrs):
    NB = v x w,  NA = u x v          (pairwise plane mults, P1-P2)
    q3 = v*v (ACT Square),  d3 = u*NB,  x3 = NA*NB
    (q, det, xx) = joint 3-group pair-sum of [q3|d3|x3]   (2 strided adds)
    yy = det * sqrt(q)  (sqrt on ACT)
    tail (direct atan2, ACT arctan saturates correctly for huge args):
        t = yy * recip(xx);  A = arctan(t)
        phi = A + (pi/2)*(sgy - sgx*sgy)
GPSIMD stays fully idle: concurrent GPSIMD tensor work inflates DVE ~2.4x
(SBUF port contention, measured).  Input DMAs on nc.sync (HWDGE), output
stores on nc.scalar.  Chunks [2,2,2,2,1,1] pipeline phase 2 against the
DMA stream with small tail chunks to bound post-last-DMA exposure.
"""

import numpy as np

import concourse.bacc as bacc
import concourse.bass as bass
import concourse.mybir as mybir
from concourse.bass_utils import run_bass_kernel_spmd
from concourse.tile import TileContext

AF = mybir.ActivationFunctionType
OP = mybir.AluOpType
F32 = mybir.dt.float32

PI = float(np.pi)

N_CORES = 8
G = 196
T = 10
Q = G * T                   # 1960 rows per partition
ROWS_PER_CORE = 128 * Q     # 250880
CHUNKS = [2, 2, 2, 2, 1, 1]

BS = 9 * G                  # b-plane block stride
SP = 20                     # scratch planes per block
SS = SP * G                 # scratch block stride


def _ap(base, off, dims):
    return bass.AP(
        base.tensor, base.offset + off, [list(base.ap[0])] + [list(d) for d in dims]
    )


def _emit_ph1(nc, bba, cpa, xta, b, c0, c1, c2, c3):
    """ACT transposing copies -> 12 carbon planes; DVE mega-sub -> 9 b-planes."""
    s, v = nc.scalar, nc.vector
    # carbons (0,1) at cols c0,c1 and (2,3) at c2,c3: uniform stride per pair
    s.activation(
        _ap(cpa, 0, [[3 * G, 2], [G, 3], [1, G]]),
        _ap(xta, c0, [[c1 - c0, 2], [1, 3], [42, G]]),
        AF.Copy,
    )
    s.activation(
        _ap(cpa, 6 * G, [[3 * G, 2], [G, 3], [1, G]]),
        _ap(xta, c2, [[c3 - c2, 2], [1, 3], [42, G]]),
        AF.Copy,
    )
    # (u,v,w) = carbons[1:4] - carbons[0:3], all contiguous
    v.tensor_tensor(
        _ap(bba, b * BS, [[1, 9 * G]]),
        _ap(cpa, 3 * G, [[1, 9 * G]]),
        _ap(cpa, 0, [[1, 9 * G]]),
        OP.subtract,
    )


def _emit_ph2(nc, bba, sca, oba, y, b0, nb):
    """Cross-form dihedral + direct atan2 for chunk [b0, b0+nb) blocks."""
    v, s = nc.vector, nc.scalar

    def bap(plane, n=1, ps=1):
        dims = [[BS, nb]] + ([[ps * G, n]] if n > 1 else []) + [[1, G]]
        return _ap(bba, b0 * BS + plane * G, dims)

    def sap(plane, n=1, ps=1):
        dims = [[SS, nb]] + ([[ps * G, n]] if n > 1 else []) + [[1, G]]
        return _ap(sca, plane * G, dims)

    # q3 = v*v -> X(0-2)
    s.activation(sap(0, 3), bap(3, 3), AF.Square)
    # NB = v x w -> C(9-11):  P1 -> C, P2 -> B(12-14), then C = C - B
    v.tensor_tensor(sap(9, 2), bap(4, 2), bap(8, 2, -2), OP.mult)   # vy*wz, vz*wx
    v.tensor_tensor(sap(12, 2), bap(5, 2, -2), bap(7, 2), OP.mult)  # vz*wy, vx*wz
    v.tensor_tensor(sap(11, 2, 3), bap(3, 2), bap(7, 2, -1), OP.mult)  # vx*wy, vy*wx
    v.tensor_tensor(sap(9, 3), sap(9, 3), sap(12, 3), OP.subtract)
    # d3 = u * NB -> X(3-5)
    v.tensor_tensor(sap(3, 3), bap(0, 3), sap(9, 3), OP.mult)
    # NA = u x v -> B(12-14): P1 -> B, P2 -> (15-17), then B = B - P2
    v.tensor_tensor(sap(12, 2), bap(1, 2), bap(5, 2, -2), OP.mult)  # uy*vz, uz*vx
    v.tensor_tensor(sap(15, 2), bap(2, 2, -2), bap(4, 2), OP.mult)  # uz*vy, ux*vz
    v.tensor_tensor(sap(14, 2, 3), bap(0, 2), bap(4, 2, -1), OP.mult)  # ux*vy, uy*vx
    v.tensor_tensor(sap(12, 3), sap(12, 3), sap(15, 3), OP.subtract)
    # x3 = NA * NB -> X(6-8)
    v.tensor_tensor(sap(6, 3), sap(12, 3), sap(9, 3), OP.mult)
    # joint sums: (q, det, xx) -> (15, 16, 17)
    v.tensor_tensor(sap(15, 3), sap(0, 3, 3), sap(1, 3, 3), OP.add)
    v.tensor_tensor(sap(15, 3), sap(15, 3), sap(2, 3, 3), OP.add)
    # yy = det * sqrt(q) -> 19
    s.activation(sap(18), sap(15), AF.Sqrt)
    v.tensor_tensor(sap(19), sap(16), sap(18), OP.mult)
    # tail: phi = arctan(yy/xx) + (pi/2)*(sgy - sgx*sgy)
    s.activation(sap(0), sap(17), AF.Sign)             # sgx
    s.activation(sap(1), sap(19), AF.Sign)             # sgy
    v.reciprocal_approx_fast(sap(2), sap(17))          # rd = 1/xx
    v.tensor_tensor(sap(3), sap(19), sap(2), OP.mult)  # t = yy * rd
    s.activation(sap(4), sap(3), AF.Arctan)            # A
    v.tensor_tensor(sap(5), sap(0), sap(1), OP.mult)   # s2 = sgx*sgy
    v.tensor_tensor(sap(5), sap(1), sap(5), OP.subtract)  # t2 = sgy - s2
    v.scalar_tensor_tensor(
        _ap(oba, 0, [[G, nb], [1, G]]), sap(5), PI / 2, sap(4), OP.mult, OP.add
    )
    nc.scalar.dma_start(
        out=y.rearrange("(p q) -> p q", p=128)[:, b0 * G : (b0 + nb) * G],
        in_=_ap(oba, 0, [[1, nb * G]]),
    )


def build_kernel(atoms):
    c0, c1, c2, c3 = (3 * int(a) for a in atoms)
    nc = bacc.Bacc("TRN2", target_bir_lowering=False, debug=False)
    x = nc.dram_tensor("x", [ROWS_PER_CORE, 42], F32, kind="ExternalInput")
    y = nc.dram_tensor("y", [ROWS_PER_CORE], F32, kind="ExternalOutput")
    xr = x.rearrange("(p q) c -> p q c", p=128)
    with TileContext(nc) as tc:
        with (
            tc.tile_pool(name="xp", bufs=2) as xp,
            tc.tile_pool(name="cp", bufs=2) as cp,
            tc.tile_pool(name="bp", bufs=1) as bp,
            tc.tile_pool(name="sp", bufs=1) as sp,
            tc.tile_pool(name="op", bufs=2) as op,
        ):
            bb = bp.tile([128, 9 * G * T], F32, tag="b")
            bba = bb[:]
            b = 0
            for nb in CHUNKS:
                for _ in range(nb):
                    xt = xp.tile([128, G * 42], F32, tag="x")
                    nc.sync.dma_start(out=xt[:], in_=xr[:, b * G : (b + 1) * G, :])
                    ct = cp.tile([128, 12 * G], F32, tag="c")
                    _emit_ph1(nc, bba, ct[:], xt[:], b, c0, c1, c2, c3)
                    b += 1
                sc = sp.tile([128, SS * 2], F32, tag="sc")
                ob = op.tile([128, G * 2], F32, tag="o")
                _emit_ph2(nc, bba, sc[:], ob[:], y, b - nb, nb)
    nc.finalize()
    return nc


_CACHE = {}


def _get_nc(atoms):
    key = tuple(int(a) for a in atoms)
    if key not in _CACHE:
        _CACHE[key] = build_kernel(key)
    return _CACHE[key]


def run(x, atoms=(0, 4, 7, 11), **spmd_kwargs):
    """x: [B, 42] f32. Returns (y [B] f32, BassKernelResults)."""
    x = np.ascontiguousarray(np.asarray(x, dtype=np.float32))
    B = x.shape[0]
    total = N_CORES * ROWS_PER_CORE
    if B < total:
        # pad with replicated leading rows (valid, non-degenerate data)
        reps = -(-(total - B) // B)
        x = np.concatenate([x] + [x] * reps, axis=0)[:total]
    nc = _get_nc(atoms)
    shards = x.reshape(N_CORES, ROWS_PER_CORE, 42)
    in_maps = [{"x": shards[i]} for i in range(N_CORES)]
    res = run_bass_kernel_spmd(nc, in_maps, core_ids=list(range(N_CORES)), **spmd_kwargs)
    y = np.concatenate([r["y"] for r in res.results])[:B]
    return np.asarray(y, dtype=np.float32), res


def kernel(x, mask_matrix):
    mask = np.asarray(mask_matrix)
    atoms = tuple(int(i) for i in np.argmax(mask, axis=1))
    y, _ = run(x, atoms=atoms)
    return y
